# revision 10
# baseline (speedup 1.0000x reference)
"""MoE expert-MLP (SwiGLU) kernel for 8 Trainium2 NeuronCores.

Strategy: expert-parallel. Each of the 8 cores owns one expert's weights.
Tokens are routed on the host: every (token, expert) routing pair is
dispatched to its expert's core, padded to a fixed per-expert capacity.
Duplicate (token, expert) slots (a token whose K=2 routing picks the same
expert twice) are merged host-side with summed weight — identical math,
and it lowers the max per-expert count from 2096 to 1996 so capacity is
2048 instead of 2176 (-6% PE rows).

All matmul operands are bf16 (PSUM accumulates in fp32): the PE runs bf16
at the same 1 cycle/row as fp32r, but every DMA stream halves, so the
whole batch fits in one pass (x^T, h^T, Wd all SBUF-resident; Wg/Wu
streamed exactly once). Measured end-to-end bf16 error is ~4e-3, well
inside the 2e-2 gate.

Per-core kernel (cap=2048 tokens, D=2048, H=1408):
  stage A: h^T[h, t] = silu(Wg @ x^T) * (Wu @ x^T)
    - d-chunk-major: for each 128-wide d chunk of x^T, issue the 4
      gate-group matmuls then the 4 up-group matmuls, so the PE consumes
      each x chunk right as its DMA lands (no pass-0 starvation).
    - PSUM: 4 banks for gate + 4 banks for up (one per 512-token group).
  stage B: y[t, d] = (h^T)^T @ Wd^T, rows scaled by the routing weight;
    psy tiles alternate between the two stage-A PSUM bank quads.
"""

import sys
import os

sys.path.insert(0, "/opt/trn_rl_repo")

import numpy as np
import ml_dtypes

BF16 = ml_dtypes.bfloat16

T, D, H, E, K = 8192, 2048, 1408, 8, 2
P = 128
HT = H // P        # 11 h-tiles
KT = D // P        # 16 d-chunks
DC = 512           # moving-dim chunk for stage B
GW = 512           # token-group width for stage A PSUM banks

_built = {}


def _pass_sizes(cap):
    """Split cap into passes of <=2048 tokens (multiples of 128)."""
    sizes = []
    rem = cap
    while rem:
        s = min(rem, 2048)
        sizes.append(s)
        rem -= s
    assert all(s % 128 == 0 for s in sizes), sizes
    return sizes


def _grp_offsets(tc):
    """Split a pass into <=4 token groups of <=512 (one PSUM bank each)."""
    offs = []
    o = 0
    while o < tc:
        g = min(GW, tc - o)
        offs.append((o, g))
        o += g
    assert len(offs) <= 4, offs
    return offs


def _build_nc(cap):
    import concourse.bass as bass  # noqa: F401
    from concourse import bacc
    import concourse.mybir as mybir
    import concourse.tile as tile

    F32 = mybir.dt.float32
    BF = mybir.dt.bfloat16
    Silu = mybir.ActivationFunctionType.Silu
    Copy = mybir.ActivationFunctionType.Copy
    Mult = mybir.AluOpType.mult

    sizes = _pass_sizes(cap)

    nc = bacc.Bacc("TRN2", target_bir_lowering=False, debug=False)
    xT = nc.declare_dram_parameter("xT", [KT, P, cap], BF, isOutput=False)
    wg = nc.declare_dram_parameter("wg", [HT, P, KT * P], BF, isOutput=False)
    wu = nc.declare_dram_parameter("wu", [HT, P, KT * P], BF, isOutput=False)
    wd = nc.declare_dram_parameter("wd", [H, D], BF, isOutput=False)
    wt = nc.declare_dram_parameter("wt", [cap], F32, isOutput=False)
    out = nc.declare_dram_parameter("out", [cap, D], BF, isOutput=True)

    with tile.TileContext(nc) as tc:
        with (
            tc.tile_pool(name="sbuf", bufs=1) as pool,
            tc.tile_pool(name="psum", bufs=1, space="PSUM") as pp,
        ):
            wd_ts = [None] * HT
            wt_t = None
            t0 = 0
            for pi, TC in enumerate(sizes):
                grps = _grp_offsets(TC)
                G = len(grps)
                # x^T chunks for this pass; one tile per d-chunk so the
                # first matmuls only wait on the first chunk's DMA. On
                # pass 0 the d=0 chunk is further sliced per token group
                # so the very first matmul waits on a 128KB transfer, not
                # 512KB; chunk enqueues are split across the GpSimd and
                # Vector queues (descriptor writes cost ~0.65us each and
                # would otherwise serialize behind one queue).
                xt_ts = []
                xt0_s = None
                for dti in range(KT):
                    if pi == 0 and dti == 0:
                        xt0_s = []
                        for j, (o, g) in enumerate(grps):
                            sl = pool.tile([P, g], BF, tag=f"xt0s{j}",
                                           bufs=1, name=f"xt0s{j}")
                            nc.gpsimd.dma_start(
                                sl[:], xT[0, :, t0 + o : t0 + o + g]
                            )
                            xt0_s.append(sl)
                        xt_ts.append(None)
                        continue
                    xt_1 = pool.tile([P, TC], BF, tag=f"xt{dti}", bufs=1,
                                     name=f"xt{dti}")
                    q = nc.gpsimd if dti <= 8 else nc.scalar
                    q.dma_start(
                        xt_1[:], xT[dti, :, t0 : t0 + TC]
                    )
                    xt_ts.append(xt_1)
                # h^T for this pass (bf16: stage-B lhs)
                h_t = pool.tile([P, HT, TC], BF, tag="ht", bufs=1)

                # ---- stage A: h^T = silu(g^T) * u^T ----
                for ht in range(HT):
                    if pi == 0 and ht == 0:
                        # slice the first weight tiles so the first
                        # matmul's LDWEIGHTS waits on a 128KB transfer
                        wg0_s = []
                        for s in range(4):
                            sl = pool.tile([P, 4 * P], BF, tag=f"wg0s{s}",
                                           bufs=1, name=f"wg0s{s}")
                            nc.sync.dma_start(
                                sl[:], wg[0, :, s * 4 * P : (s + 1) * 4 * P]
                            )
                            wg0_s.append(sl)
                        wu0_h = []
                        for s in range(2):
                            sl = pool.tile([P, 8 * P], BF, tag=f"wu0h{s}",
                                           bufs=1, name=f"wu0h{s}")
                            nc.sync.dma_start(
                                sl[:], wu[0, :, s * 8 * P : (s + 1) * 8 * P]
                            )
                            wu0_h.append(sl)
                        wg_t = wu_t = None
                    else:
                        wg0_s = wu0_h = None
                        wg_t = pool.tile([P, KT * P], BF, tag="wgu", bufs=4)
                        nc.sync.dma_start(wg_t[:], wg[ht, :, :])
                        wu_t = pool.tile([P, KT * P], BF, tag="wgu", bufs=4)
                        nc.sync.dma_start(wu_t[:], wu[ht, :, :])
                    if pi == 0 and ht == 1:
                        # stream Wd + wt once x (the ht==0 stream) is done
                        # so they don't contend with x for Q0 bandwidth
                        for hh in range(HT):
                            wdc = pool.tile([P, D], BF, tag=f"wd{hh}",
                                            bufs=1, name=f"wdc{hh}")
                            nc.gpsimd.dma_start(
                                wdc[:], wd[hh * P : (hh + 1) * P, :]
                            )
                            wd_ts[hh] = wdc
                        wt_t = pool.tile([P, cap // P], F32, tag="wt",
                                         bufs=1)
                        nc.gpsimd.dma_start(
                            wt_t[:], wt.rearrange("(n p) -> p n", p=P)
                        )

                    psg = [pp.tile([P, g], F32, tag="pa", bufs=4,
                                   name=f"psg{j}") for j, (o, g) in
                           enumerate(grps)]
                    psu = [pp.tile([P, g], F32, tag="pb", bufs=4,
                                   name=f"psu{j}") for j, (o, g) in
                           enumerate(grps)]
                    st_ts = [None] * G

                    def rhs(d, j, o, g):
                        if d == 0 and xt0_s is not None:
                            return xt0_s[j][:]
                        return xt_ts[d][:, o : o + g]

                    for d in range(KT):
                        if wg0_s is not None:
                            lhs_g = wg0_s[d // 4][:, (d % 4) * P : (d % 4 + 1) * P]
                        else:
                            lhs_g = wg_t[:, d * P : (d + 1) * P]
                        for j, (o, g) in enumerate(grps):
                            nc.tensor.matmul(
                                psg[j][:],
                                lhs_g,
                                rhs(d, j, o, g),
                                start=(d == 0),
                                stop=(d == KT - 1),
                            )
                        if d == KT - 1:
                            # silu runs on Scalar behind the remaining
                            # up-chain matmuls
                            for j, (o, g) in enumerate(grps):
                                st = pool.tile([P, g], F32, tag="silu",
                                               bufs=4, name="st")
                                nc.scalar.activation(st[:], psg[j][:], Silu)
                                st_ts[j] = st
                        if wu0_h is not None:
                            lhs_u = wu0_h[d // 8][:, (d % 8) * P : (d % 8 + 1) * P]
                        else:
                            lhs_u = wu_t[:, d * P : (d + 1) * P]
                        for j, (o, g) in enumerate(grps):
                            nc.tensor.matmul(
                                psu[j][:],
                                lhs_u,
                                rhs(d, j, o, g),
                                start=(d == 0),
                                stop=(d == KT - 1),
                            )
                    for j, (o, g) in enumerate(grps):
                        nc.vector.tensor_tensor(
                            h_t[:, ht, o : o + g],
                            st_ts[j][:],
                            psu[j][:],
                            op=Mult,
                        )

                # ---- stage B: y = h @ Wd^T, scaled by routing weight ----
                for ts_ in range(TC // P):
                    tagb = "pa" if ts_ % 2 == 0 else "pb"
                    psy = [pp.tile([P, DC], F32, tag=tagb, bufs=4,
                                   name=f"psy{i}") for i in range(4)]
                    for ht in range(HT):
                        lhs = h_t[:, ht, ts_ * P : (ts_ + 1) * P]
                        for dc in range(4):
                            nc.tensor.matmul(
                                psy[dc][:],
                                lhs,
                                wd_ts[ht][:, dc * DC : (dc + 1) * DC],
                                start=(ht == 0),
                                stop=(ht == HT - 1),
                            )
                    col = t0 // P + ts_
                    # drain alternates Vector / Scalar (Copy with AP
                    # scale) so the final tile's 4 scales run in parallel
                    # pairs; each quarter DMAs out on its own.
                    for dc in range(4):
                        y_q = pool.tile([P, DC], BF, tag="yout", bufs=4,
                                        name="y_q")
                        if dc % 2 == 0:
                            nc.vector.tensor_scalar_mul(
                                y_q[:], psy[dc][:], wt_t[:, col : col + 1]
                            )
                        else:
                            nc.scalar.activation(
                                y_q[:], psy[dc][:], Copy,
                                scale=wt_t[:, col : col + 1],
                            )
                        nc.sync.dma_start(
                            out[
                                t0 + ts_ * P : t0 + (ts_ + 1) * P,
                                dc * DC : (dc + 1) * DC,
                            ],
                            y_q[:],
                        )
                t0 += TC

    nc.finalize()
    return nc


def _get_nc(cap):
    if cap not in _built:
        _built[cap] = _build_nc(cap)
    return _built[cap]


def kernel(x, weights, Wg, Wu, Wd, indices, seq_len=None, **_unused):
    from concourse.bass_utils import run_bass_kernel_spmd

    x = np.asarray(x, dtype=np.float32)
    weights = np.asarray(weights, dtype=np.float32)
    Wg = np.asarray(Wg, dtype=np.float32)
    Wu = np.asarray(Wu, dtype=np.float32)
    Wd = np.asarray(Wd, dtype=np.float32)
    indices = np.asarray(indices)

    t, d = x.shape
    e, h, _ = Wg.shape
    k = indices.shape[1]

    # ---- host-side routing (dispatch) ----
    # Merge duplicate (token, expert) slots: scatter-add semantics mean a
    # token routed twice to the same expert contributes (w1+w2)*MLP_e(x).
    flat_e = indices.reshape(-1).astype(np.int64)
    flat_t = np.repeat(np.arange(t, dtype=np.int64), k)
    keys = flat_e * t + flat_t
    uniq, inv = np.unique(keys, return_inverse=True)
    uw = np.bincount(inv, weights=weights.reshape(-1).astype(np.float64))
    uw = uw.astype(np.float32)
    u_e = (uniq // t).astype(np.int64)
    u_t = (uniq % t).astype(np.int64)

    counts = np.bincount(u_e, minlength=e)
    starts = np.zeros(e + 1, dtype=np.int64)
    starts[1:] = np.cumsum(counts)
    cap = int(-(-max(int(counts.max()), 512) // P) * P)

    in_maps = []
    for ei in range(e):
        n = int(counts[ei])
        toks = u_t[starts[ei] : starts[ei] + n]
        xe = np.zeros((cap, d), dtype=np.float32)
        xe[:n] = x[toks]
        wvec = np.zeros(cap, dtype=np.float32)
        wvec[:n] = uw[starts[ei] : starts[ei] + n]
        # x^T in 128-row d-chunks, each contiguous for one DMA
        xTb = np.ascontiguousarray(xe.astype(BF16).T).reshape(KT, P, cap)
        # pack Wg/Wu so each h-tile block is one contiguous [128, 2048] DMA:
        # block[ht][p][k*128+hh] = Wg[e].T[k*128+p, ht*128+hh]
        WgT = Wg[ei].T.astype(BF16)  # [D, H]
        WuT = Wu[ei].T.astype(BF16)
        wg_lin = np.ascontiguousarray(
            WgT.reshape(KT, P, HT, P).transpose(2, 1, 0, 3).reshape(HT, P, KT * P)
        )
        wu_lin = np.ascontiguousarray(
            WuT.reshape(KT, P, HT, P).transpose(2, 1, 0, 3).reshape(HT, P, KT * P)
        )
        wdT = np.ascontiguousarray(Wd[ei].T.astype(BF16))  # [H, D]
        in_maps.append(
            {
                "xT": xTb,
                "wg": wg_lin,
                "wu": wu_lin,
                "wd": wdT,
                "wt": wvec,
            }
        )

    nc = _get_nc(cap)
    trace = bool(int(os.environ.get("KERNEL_TRACE", "0")))
    res = run_bass_kernel_spmd(
        nc, in_maps, core_ids=list(range(e)), trace=trace
    )
    if trace:
        kernel.last_exec_time_ns = res.exec_time_ns
        kernel.last_results = res

    # ---- host-side combine (scatter-add over unique slots) ----
    y = np.zeros((t, d), dtype=np.float32)
    for ei in range(e):
        n = int(counts[ei])
        toks = u_t[starts[ei] : starts[ei] + n]
        np.add.at(y, toks, res.results[ei]["out"][:n].astype(np.float32))
    return y
